# revision 55
# baseline (speedup 1.0000x reference)
"""Trainium2 Bass kernel for nn_Attention (LN + FiLM + MHA with ALiBi + entropy loss).

Sharding: 8 cores = 4 batches x 2 head-pairs. Core c handles batch c//2,
heads [2*(c%2), 2*(c%2)+1]. Everything on-device runs in "transposed" layout
(scores S^T with keys j on partitions, queries i on the free axis) so the
attention AV matmul needs no transposes; softmax/entropy j-reductions are
ones-vector matmuls on the TensorEngine. The FiLM (gamma/beta) and LN affine
are folded into the QKV weights on the host; per-row normalization 1/Z and
the final entropy assembly (log Z - Se/Z) happen on the host from per-row
stats returned by the kernel.
"""

import sys

sys.path.insert(0, "/opt/trn_rl_repo")

import numpy as np

import concourse.bass as bass
import concourse.tile as tile
from concourse import bacc
from concourse import mybir
from concourse.bass_utils import run_bass_kernel_spmd

F32 = mybir.dt.float32
F32R = mybir.dt.float32r
BF16 = mybir.dt.bfloat16
AF = mybir.ActivationFunctionType
ALU = mybir.AluOpType

N = 2048
D = 512
DH = 128
NH2 = 2  # local heads per core
NIB = N // 512  # 4 i-blocks (queries, free axis)
NJC = N // 128  # 16 j-chunks (keys, partition axis)
NDC = D // 128  # 4 d-chunks
OUT_ROWS = N + 16  # 2048 partial-out rows + 8 Z rows + 8 Se rows


def r32(ap):
    return ap


def build_nc():
    nc = bacc.Bacc("TRN2")
    xt = nc.declare_dram_parameter("xt", [D, N], F32R, False)[:, :]
    alibit = nc.declare_dram_parameter("alibit", [N, N], BF16, False)[:, :]
    wqkv = nc.declare_dram_parameter("wqkv", [D, 3 * 256], F32R, False)[:, :]
    wout = nc.declare_dram_parameter("wout", [256, D], F32R, False)[:, :]
    consts = nc.declare_dram_parameter("consts", [128, 2185], F32R, False)[:, :]
    identb = nc.declare_dram_parameter("identb", [128, 130], BF16, False)[:, :]
    out_t = nc.declare_dram_parameter("out", [OUT_ROWS, D], F32, True)[:, :]
    mu_scr = nc.dram_tensor("mu_scr", [4, 512], F32, kind="Internal")[:, :]
    r_scr = nc.dram_tensor("r_scr", [4, 512], F32, kind="Internal")[:, :]
    rz_scr = nc.dram_tensor("rz_scr", [8, 512], F32, kind="Internal")[:, :]

    def bcast(ap_row, parts=128):
        # replicate a [1, W] DRAM row across `parts` partitions
        return bass.AP(
            tensor=ap_row.tensor, offset=ap_row.offset, ap=[[0, parts]] + list(ap_row.ap[1:])
        )

    with tile.TileContext(nc) as tc:
        with tc.tile_pool(name="persist", bufs=1) as P1:
            cst_sb = P1.tile([128, 2185], F32R)
            nc.sync.dma_start(out=cst_sb, in_=consts)
            identb_full = P1.tile([128, 130], BF16)
            nc.sync.dma_start(out=identb_full, in_=identb)
            xt_sb = P1.tile([128, NDC, N], F32R)
            xt_r = xt.rearrange("(c p) n -> p c n", p=128)
            for ib in range(NIB):
                isl = slice(ib * 512, (ib + 1) * 512)
                for dc in range(NDC):
                    nc.sync.dma_start(out=xt_sb[:, dc, isl], in_=xt_r[:, dc, isl])
            wq_sb = P1.tile([128, NDC, 768], F32R)
            nc.sync.dma_start(out=wq_sb, in_=wqkv.rearrange("(c p) m -> p c m", p=128))
            wo_sb = P1.tile([128, 2, D], F32R)
            nc.sync.dma_start(out=wo_sb, in_=wout.rearrange("(c p) m -> p c m", p=128))
            identb_sb = identb_full[:, 0:128]
            ones_b = identb_full[:, 128:129]
            ones_col = cst_sb[:, 128:129]
            bias_vb = cst_sb[:, 905:1161].bitcast(F32)  # replicated v bias rows
            qT = P1.tile([128, NH2, N], F32R)  # pre-scaled q^T per local head
            kT = P1.tile([128, NH2, N], F32R)
            v_sb = P1.tile([128, NJC, 256], BF16)  # [j, jc, (v_h0|v_h1)]
            oT = P1.tile([128, NH2, N], F32R)  # normalized attention out^T
            rb = P1.tile([128, NIB, 512], F32)   # rstd broadcast over partitions
            mut = P1.tile([128, 16], F32)        # mu transposed (j on partitions)
            rt = P1.tile([128, 16], F32)

            # ---------------- Phase 1: LayerNorm stats (per-ib pipelined) ----
            with (
                tc.tile_pool(name="ln_sb", bufs=1) as LNP,
                tc.tile_pool(name="ln_sq", bufs=4) as LNQ,
                tc.tile_pool(name="ln_ps", bufs=2, space="PSUM") as LNPS,
            ):
                mu = LNP.tile([1, N], F32R, tag="mu")
                m2 = LNP.tile([1, N], F32, tag="m2")
                rstd = LNP.tile([1, N], F32R, tag="rstd")
                musq = LNP.tile([1, N], F32, tag="musq")
                sq = LNP.tile([1, N], F32R, tag="sq")
                eps_t = LNP.tile([1, 1], F32, tag="eps")
                nc.vector.memset(eps_t, 1e-5)
                for ib in range(NIB):
                    sl = slice(ib * 512, (ib + 1) * 512)
                    ps = LNPS.tile([1, 512], F32, tag="stat")
                    for dc in range(NDC):
                        nc.tensor.matmul(
                            ps, lhsT=r32(ones_col), rhs=r32(xt_sb[:, dc, sl]),
                            start=(dc == 0), stop=(dc == NDC - 1),
                        )
                    nc.scalar.mul(out=mu[:, sl], in_=ps, mul=1.0 / D)
                    ps2 = LNPS.tile([1, 512], F32, tag="stat")
                    for dc in range(NDC):
                        xsq_t = LNQ.tile([128, 512], F32R, tag="xsq")
                        nc.scalar.activation(out=xsq_t, in_=xt_sb[:, dc, sl], func=AF.Square)
                        nc.tensor.matmul(
                            ps2, lhsT=r32(ones_col), rhs=r32(xsq_t),
                            start=(dc == 0), stop=(dc == NDC - 1),
                        )
                    nc.scalar.mul(out=m2[:, sl], in_=ps2, mul=1.0 / D)
                    # var = m2 - mu^2; rstd = rsqrt(var+eps); rm = mu*rstd
                    mu_f = mu.bitcast(F32)
                    nc.vector.tensor_mul(musq[:, sl], mu_f[:, sl], mu_f[:, sl])
                    nc.vector.tensor_sub(m2[:, sl], m2[:, sl], musq[:, sl])
                    nc.scalar.activation(out=sq[:, sl], in_=m2[:, sl], func=AF.Sqrt, bias=eps_t)
                    with nc.allow_low_precision(reason="rstd broadcast via f32r matmul"):
                        nc.vector.reciprocal(rstd[:, sl], sq.bitcast(F32)[:, sl])
                    # bounce this 512-strip, broadcast + transposed gathers
                    nc.sync.dma_start(out=mu_scr[ib : ib + 1, :], in_=mu_f[:, sl])
                    nc.sync.dma_start(out=r_scr[ib : ib + 1, :], in_=rstd.bitcast(F32)[:, sl])
                    nc.sync.dma_start(
                        out=rb[:, ib, :],
                        in_=bass.AP(tensor=r_scr.tensor, offset=ib * 512, ap=[[0, 128], [1, 512]]),
                    )
                    nc.sync.dma_start(
                        out=mut[:, ib * 4 : ib * 4 + 4],
                        in_=bass.AP(tensor=mu_scr.tensor, offset=ib * 512, ap=[[1, 128], [128, 4]]),
                    )
                    nc.sync.dma_start(
                        out=rt[:, ib * 4 : ib * 4 + 4],
                        in_=bass.AP(tensor=r_scr.tensor, offset=ib * 512, ap=[[1, 128], [128, 4]]),
                    )

                # ------------ Phase 2: QKV projections (fused LN fixups) -----
                with tc.tile_pool(name="qk_ps", bufs=3, space="PSUM") as QPS:
                    for jc in range(NJC):
                        jsl = slice(jc * 128, (jc + 1) * 128)
                        pv = QPS.tile([128, 256], F32, tag="v")
                        for dc in range(NDC):
                            nc.tensor.matmul(
                                pv, lhsT=r32(xt_sb[:, dc, jsl]), rhs=r32(wq_sb[:, dc, 512:768]),
                                start=(dc == 0), stop=False,
                            )
                        nc.tensor.matmul(
                            pv, lhsT=mu[:, jsl], rhs=cst_sb[0:1, 649:905],
                            start=False, stop=True,
                        )
                        v_ap = v_sb[:, jc, :]
                        nc.vector.scalar_tensor_tensor(
                            out=v_ap, in0=pv, scalar=rt[:, jc : jc + 1],
                            in1=bias_vb, op0=ALU.mult, op1=ALU.add,
                        )
                    for h2 in range(NH2):
                        qcols = slice(h2 * 128, (h2 + 1) * 128)
                        kcols = slice(256 + h2 * 128, 256 + (h2 + 1) * 128)
                        for bi_c, cols, dst in ((1, kcols, kT), (0, qcols, qT)):
                            for ib in range(NIB):
                                sl = slice(ib * 512, (ib + 1) * 512)
                                pq = QPS.tile([128, 512], F32, tag="qk")
                                for dc in range(NDC):
                                    nc.tensor.matmul(
                                        pq, lhsT=r32(wq_sb[:, dc, cols]), rhs=r32(xt_sb[:, dc, sl]),
                                        start=(dc == 0), stop=False,
                                    )
                                seg = 2 * bi_c + h2
                                ws_row = cst_sb[0:1, 1161 + seg * 128 : 1161 + (seg + 1) * 128]
                                nc.tensor.matmul(
                                    pq, lhsT=ws_row, rhs=mu[:, sl], start=False, stop=False,
                                )
                                b_row = cst_sb[0:1, 1673 + seg * 128 : 1673 + (seg + 1) * 128]
                                nc.tensor.matmul(
                                    pq, lhsT=b_row, rhs=sq[:, sl], start=False, stop=True,
                                )
                                nc.vector.tensor_mul(dst[:, h2, sl], pq, rb[:, ib, :])

            # ---------------- Phase 3: attention (transposed scores) --------
            with (
                tc.tile_pool(name="s_ps", bufs=3, space="PSUM") as SS,
                tc.tile_pool(name="at_ps", bufs=2, space="PSUM") as APS,
                tc.tile_pool(name="st_ps", bufs=1, space="PSUM") as SPS,
                tc.tile_pool(name="at_sb", bufs=12) as ASB,
                tc.tile_pool(name="ep_sb", bufs=4) as EP,
            ):
                def emit_outproj(ib_op):
                    for i1 in range(ib_op * 4, ib_op * 4 + 4):
                        osl = slice(i1 * 128, (i1 + 1) * 128)
                        pp = SPS.tile([128, 512], F32, tag="p")
                        nc.tensor.matmul(
                            pp, lhsT=r32(oT[:, 0, osl]), rhs=r32(wo_sb[:, 0, :]),
                            start=True, stop=False,
                        )
                        nc.tensor.matmul(
                            pp, lhsT=r32(oT[:, 1, osl]), rhs=r32(wo_sb[:, 1, :]),
                            start=False, stop=True,
                        )
                        p_sb = EP.tile([128, 512], F32, tag="psb")
                        nc.any.tensor_copy(out=p_sb, in_=pp)
                        nc.sync.dma_start(out=out_t[osl, :], in_=p_sb)

                pending_op = [None]
                for h2 in range(NH2):
                    vcols = slice(h2 * 128, (h2 + 1) * 128)
                    for ib in range(NIB):
                        isl = slice(ib * 512, (ib + 1) * 512)
                        if pending_op[0] is not None:
                            emit_outproj(pending_op[0])
                            pending_op[0] = None
                        po = APS.tile([128, 512], F32, tag="o")
                        pz = SPS.tile([1, 512], F32, tag="z")
                        pse_t = SPS.tile([33, 512], F32, tag="se")
                        pse = pse_t[32:33, :]
                        prev = None
                        for jc in range(NJC):
                            jsl = slice(jc * 128, (jc + 1) * 128)
                            al = ASB.tile([128, 512], BF16, tag="al")
                            nc.sync.dma_start(out=al, in_=alibit[jsl, isl])
                            ps = SS.tile([128, 512], F32, tag="s")
                            nc.tensor.matmul(
                                ps, lhsT=r32(kT[:, h2, jsl]), rhs=r32(qT[:, h2, isl]),
                                start=True, stop=False,
                            )
                            nc.tensor.matmul(
                                ps, lhsT=identb_sb, rhs=al, start=False, stop=True
                            )
                            e = ASB.tile([128, 512], BF16, tag="e")
                            nc.scalar.activation(out=e, in_=ps, func=AF.Exp)
                            et = ASB.tile([128, 512], BF16, tag="et")
                            nc.vector.tensor_mul(et, e, ps)
                            if prev is not None:
                                pe_, pet_, pjc = prev
                                nc.tensor.matmul(
                                    po, lhsT=v_sb[:, pjc, vcols], rhs=pe_,
                                    start=(pjc == 0), stop=False,
                                )
                                nc.tensor.matmul(
                                    pz, lhsT=ones_b, rhs=pe_,
                                    start=(pjc == 0), stop=False,
                                    tile_position=(0, 0),
                                )
                                nc.tensor.matmul(
                                    pse, lhsT=ones_b, rhs=pet_,
                                    start=(pjc == 0), stop=False,
                                    tile_position=(0, 32),
                                )
                            prev = (e, et, jc)
                        pe_, pet_, pjc = prev
                        nc.tensor.matmul(
                            po, lhsT=v_sb[:, pjc, vcols], rhs=pe_,
                            start=False, stop=True,
                        )
                        nc.tensor.matmul(
                            pz, lhsT=ones_b, rhs=pe_,
                            start=False, stop=True,
                            tile_position=(0, 0),
                        )
                        nc.tensor.matmul(
                            pse, lhsT=ones_b, rhs=pet_,
                            start=False, stop=True,
                            tile_position=(0, 32),
                        )
                        # epilogue: stats out, 1/Z broadcast, normalize O^T
                        z_sb = EP.tile([1, 512], F32, tag="zs")
                        nc.any.tensor_copy(out=z_sb, in_=pz)
                        r_out = N + h2 * 4 + ib
                        nc.sync.dma_start(out=out_t[r_out : r_out + 1, :], in_=z_sb)
                        se_sb = EP.tile([1, 512], F32, tag="ses")
                        nc.any.tensor_copy(out=se_sb, in_=pse)
                        r_out2 = N + 8 + h2 * 4 + ib
                        nc.sync.dma_start(out=out_t[r_out2 : r_out2 + 1, :], in_=se_sb)
                        rz = EP.tile([1, 512], F32R, tag="rz")
                        with nc.allow_low_precision(reason="1/Z broadcast via f32r matmul"):
                            nc.vector.reciprocal(rz, z_sb)
                        pb = SPS.tile([128, 512], F32, tag="p")
                        nc.tensor.matmul(
                            pb, lhsT=cst_sb[0:1, 129:257], rhs=rz, start=True, stop=True
                        )
                        rzb = EP.tile([128, 512], F32, tag="rzb")
                        nc.any.tensor_copy(out=rzb, in_=pb)
                        nc.vector.tensor_mul(oT[:, h2, isl], po, rzb)
                        if h2 == 1:
                            pending_op[0] = ib
                if pending_op[0] is not None:
                    emit_outproj(pending_op[0])
    return nc


_NC_CACHE = None


def _get_nc():
    global _NC_CACHE
    if _NC_CACHE is None:
        _NC_CACHE = build_nc()
        if not _NC_CACHE.is_finalized():
            _NC_CACHE.finalize()
    return _NC_CACHE


def _mk_consts(bq, bk, bv, wq, wk, wv):
    c = np.zeros((128, 2185), dtype=np.float32)
    c[:, :128] = np.eye(128, dtype=np.float32)
    c[:, 128:641] = 1.0
    c[:, 641] = bq[:128]
    c[:, 642] = bq[128:]
    c[:, 643] = bk[:128]
    c[:, 644] = bk[128:]
    c[:, 645] = -wq.sum(axis=0)[:128]
    c[:, 646] = -wq.sum(axis=0)[128:]
    c[:, 647] = -wk.sum(axis=0)[:128]
    c[:, 648] = -wk.sum(axis=0)[128:]
    c[:, 649:905] = -wv.sum(axis=0)[None, :]
    c[:, 905:1161] = bv[None, :]
    c[0, 1161:1289] = -wq.sum(axis=0)[:128]
    c[0, 1289:1417] = -wq.sum(axis=0)[128:]
    c[0, 1417:1545] = -wk.sum(axis=0)[:128]
    c[0, 1545:1673] = -wk.sum(axis=0)[128:]
    c[0, 1673:1801] = bq[:128]
    c[0, 1801:1929] = bq[128:]
    c[0, 1929:2057] = bk[:128]
    c[0, 2057:2185] = bk[128:]
    return c


def _prep_in_maps(x, alibi_bias, gamma, beta, ln_w, ln_b, w_qkv, w_out):
    scale = DH ** -0.5
    in_maps = []
    alibit_cache = {}
    import ml_dtypes
    identb = np.zeros((128, 130), dtype=ml_dtypes.bfloat16)
    identb[:, :128] = np.eye(128, dtype=ml_dtypes.bfloat16)
    identb[:, 128:] = 1.0
    for c in range(8):
        bi, hp = c // 2, c % 2
        h0 = 2 * hp
        g = (gamma[bi] * ln_w).astype(np.float32)  # [512]
        be = (gamma[bi] * ln_b + beta[bi]).astype(np.float32)
        qc = slice(h0 * DH, (h0 + 2) * DH)
        kc = slice(D + h0 * DH, D + (h0 + 2) * DH)
        vc = slice(2 * D + h0 * DH, 2 * D + (h0 + 2) * DH)
        wq = g[:, None] * w_qkv[:, qc] * scale
        wk = g[:, None] * w_qkv[:, kc]
        wv = g[:, None] * w_qkv[:, vc]
        bq = (be @ w_qkv[:, qc]) * scale
        bk = be @ w_qkv[:, kc]
        bv = be @ w_qkv[:, vc]
        if bi not in alibit_cache:
            alibit_cache[bi] = np.ascontiguousarray(alibi_bias[bi].T.astype(ml_dtypes.bfloat16))
        in_maps.append(
            {
                "xt": np.ascontiguousarray(x[bi].T),
                "identb": identb,
                "alibit": alibit_cache[bi],
                "wqkv": np.ascontiguousarray(
                    np.concatenate([wq, wk, wv], axis=1).astype(np.float32)
                ),
                "wout": np.ascontiguousarray(w_out[h0 * DH : (h0 + 2) * DH, :]).astype(
                    np.float32
                ),
                "consts": _mk_consts(bq, bk, bv, wq, wk, wv),
            }
        )
    return in_maps


def _postprocess(results):
    out = np.zeros((4, N, D), dtype=np.float32)
    ent_sum = 0.0
    for c in range(8):
        r = np.asarray(results[c]["out"])
        out[c // 2] += r[:N]
        for h2 in range(NH2):
            z = r[N + h2 * 4 : N + h2 * 4 + 4].reshape(-1).astype(np.float64)
            se = r[N + 8 + h2 * 4 : N + 8 + h2 * 4 + 4].reshape(-1).astype(np.float64)
            ent_sum += float(np.sum(np.log(z) - se / z))
    loss = np.float32(0.1 * ent_sum / (4 * 4 * N))
    return out, loss


def kernel(x, alibi_bias, gamma, beta, ln_w, ln_b, w_qkv, w_out, trace=False):
    nc = _get_nc()
    in_maps = _prep_in_maps(
        np.asarray(x, np.float32),
        np.asarray(alibi_bias, np.float32),
        np.asarray(gamma, np.float32),
        np.asarray(beta, np.float32),
        np.asarray(ln_w, np.float32),
        np.asarray(ln_b, np.float32),
        np.asarray(w_qkv, np.float32),
        np.asarray(w_out, np.float32),
    )
    res = run_bass_kernel_spmd(nc, in_maps, core_ids=list(range(8)), trace=trace)
    out, loss = _postprocess(res.results)
    if trace:
        kernel.last_exec_time_ns = res.exec_time_ns
    return out, loss


kernel.last_exec_time_ns = None
